# revision 27
# baseline (speedup 1.0000x reference)
"""AlgebraicTransformerLM on 8 trn2 NeuronCores (Bass/Tile), v3.

DP=2 over batch x TP=4 over heads / d_ffn / vocab. All matmul operands
fp16; ALiBi via integer-index aug rows (exact in fp16). The residual
stream lives as bf16 [d, 512-token-strip] tiles that flow THROUGH the
AllReduce: each core contributes delta + x/4, so the collective output
IS the updated residual and lands by plain DMA (no add ops, no engine
blocking). Emission is interleaved via generators so the elementwise-
bound score phases always have independent matmuls queued behind them.
"""
import contextlib
import math

import numpy as np

import concourse.bacc as bacc
import concourse.mybir as mybir
import concourse.tile as tile
from concourse.bass_utils import run_bass_kernel_spmd

F32 = mybir.dt.float32
F16 = mybir.dt.float16
BF16 = mybir.dt.bfloat16
AF = mybir.ActivationFunctionType
ALU = mybir.AluOpType

B, T, V, D, H, L = 2, 1024, 32000, 1024, 16, 4
DFF = 2730
DH = D // H
SCALE = 1.0 / math.sqrt(DH)
EPS = 1e-6

NCORES = 8
TP = 4
HPC = H // TP               # heads per core (4)
FSH = 2 * DH * HPC          # q+k rows per core (512)
VSH = DH * HPC              # v rows per core (256)
DFF_SH = 768                # padded DFF shard (4*768 >= 2730)
NFT_FF = DFF_SH // 128      # 6
VOC_SH = V // TP            # vocab shard per core (8000)
DT = D // 128               # 8
NSTRIP = T // 512           # 2
RG = [[0, 1, 2, 3], [4, 5, 6, 7]]
ALIBI = [2.0 ** (-8.0 * (i + 1) / H) for i in range(H)]

_CACHE = {}


def _causal_tk(s):
    return list(range((s + 1) * (512 // 128)))


def _mask_base(tk, s):
    base = s * 512 - tk * 128
    return base if tk * 128 + 127 > s * 512 else None


def _run(gen):
    for _ in gen:
        pass


def _weave(*gens):
    live = list(gens)
    while live:
        for g in list(live):
            try:
                next(g)
            except StopIteration:
                live.remove(g)


def build_nc():
    nc = bacc.Bacc("TRN2", target_bir_lowering=False)

    x0T = nc.dram_tensor("x0T", [D, T], BF16, kind="ExternalInput")
    qaug = nc.dram_tensor("qaug", [HPC, 2, T], F16, kind="ExternalInput")
    kaug = nc.dram_tensor("kaug", [HPC, 2, T], F16, kind="ExternalInput")
    wqkT = nc.dram_tensor("wqkT", [L, D, FSH], F16, kind="ExternalInput")
    wvT = nc.dram_tensor("wvT", [L, D, VSH], F16, kind="ExternalInput")
    woT = nc.dram_tensor("woT", [L, VSH, D], F16, kind="ExternalInput")
    wmT = nc.dram_tensor("wmT", [L, D, 2 * DFF_SH], F16, kind="ExternalInput")
    w3T = nc.dram_tensor("w3T", [L, DFF_SH, D], F16, kind="ExternalInput")
    membT = nc.dram_tensor("membT", [D, VOC_SH], F16, kind="ExternalInput")
    logits = nc.dram_tensor("logits", [T, VOC_SH], F32, kind="ExternalOutput")
    NCC = 2 * L * NSTRIP
    cc_in = [nc.dram_tensor(f"cc_in{i}", [D, 512], BF16) for i in range(NCC)]
    cc_out = [nc.dram_tensor(f"cc_out{i}", [D, 512], BF16) for i in range(NCC)]
    ccw_in = nc.dram_tensor("ccw_in", [1, 512], BF16)
    ccw_out = nc.dram_tensor("ccw_out", [1, 512], BF16)

    with tile.TileContext(nc) as tc, contextlib.ExitStack() as ctx:
        persist = ctx.enter_context(tc.tile_pool(name="persist", bufs=1))
        psA = ctx.enter_context(tc.tile_pool(name="psA", bufs=2, space="PSUM"))
        psSc = ctx.enter_context(tc.tile_pool(name="psSc", bufs=3, space="PSUM"))
        psAcc = ctx.enter_context(tc.tile_pool(name="psAcc", bufs=2, space="PSUM"))
        psSm = ctx.enter_context(tc.tile_pool(name="psSm", bufs=1, space="PSUM"))

        xn = persist.tile([128, DT, T], F16, tag="xn")

        of = persist.tile([1, 128], F32, tag="ones_f")
        nc.vector.memset(of[:], 1.0)
        ones_st = persist.tile([1, 128], F16, tag="ones_st")
        nc.vector.tensor_copy(ones_st[:], of[:])
        ocf = persist.tile([128, 1], F32, tag="ones_colf")
        nc.vector.memset(ocf[:], 1.0)
        ones_col = persist.tile([128, 1], F16, tag="ones_col")
        nc.vector.tensor_copy(ones_col[:], ocf[:])
        ones_b = persist.tile([128, 1], F32, tag="ones_bias")
        nc.vector.memset(ones_b[:], 1.0)
        eps16 = persist.tile([1, 1], F32, tag="eps16")
        nc.vector.memset(eps16[:], 16.0 * EPS)
        ccw = persist.tile([1, 512], BF16, tag="ccw")
        nc.vector.memset(ccw[:], 0.0)
        nc.sync.dma_start(ccw_in[:], ccw[:])
        nc.gpsimd.collective_compute("AllReduce", ALU.add, ins=[ccw_in[:]],
                                     outs=[ccw_out[:]], replica_groups=RG)

        # residual stream: one bf16 [128, DT, 512] tile per strip, updated
        # through the collectives. xb[s] is the CURRENT tile for strip s.
        xb = [None, None]

        with tc.tile_pool(name="xpool", bufs=3) as xpool, \
             tc.tile_pool(name="wpool", bufs=1) as wpool, \
             tc.tile_pool(name="wvpool", bufs=1) as wvpool, \
             tc.tile_pool(name="wopool", bufs=1) as wopool, \
             tc.tile_pool(name="wmpool", bufs=1) as wmpool, \
             tc.tile_pool(name="w3pool", bufs=1) as w3pool, \
             tc.tile_pool(name="apool", bufs=1) as apool, \
             tc.tile_pool(name="hpool", bufs=1) as hpool, \
             tc.tile_pool(name="qkpool", bufs=1) as qkpool, \
             tc.tile_pool(name="dpool", bufs=2) as dpool, \
             tc.tile_pool(name="scrpool", bufs=5) as scrpool, \
             tc.tile_pool(name="w4pool", bufs=2) as w4pool, \
             tc.tile_pool(name="rowpool", bufs=2) as rowpool, \
             tc.tile_pool(name="lmw", bufs=2) as lmw, \
             tc.tile_pool(name="lms", bufs=2) as lms:

            class SPool:
                _n = 0

                def tile(self, shape, dtype, tag):
                    SPool._n += 1
                    pool = {"scr": scrpool, "w4": w4pool, "xa": w4pool,
                            "row": rowpool, "rowh": rowpool}[tag]
                    return pool.tile(shape, dtype, tag=tag,
                                     name=f"{tag}_{SPool._n}")

            sp = SPool()
            asb = apool.tile([128, 2, T], F16, tag="asb")

            for s in range(NSTRIP):
                xi = xpool.tile([128, DT, 512], BF16, tag="xs",
                                name=f"x_init{s}")
                nc.sync.dma_start(
                    xi[:], x0T[:, s * 512:(s + 1) * 512]
                    .rearrange("(dt p) t -> p dt t", p=128))
                xb[s] = xi

            def recv_x(idx, s):
                """Updated residual strip arrives from the collective."""
                xs = xpool.tile([128, DT, 512], BF16, tag="xs",
                                name=f"x_{idx}")
                nc.sync.dma_start(
                    xs[:], cc_out[idx][:].rearrange("(dt p) t -> p dt t",
                                                    p=128))
                xb[s] = xs

            def norm_strip(s):
                """xn[:, :, strip] = x / (mean|x| + eps), fp16."""
                sl = slice(s * 512, (s + 1) * 512)
                xs = xb[s]
                mags = psSm.tile([1, 512], F32, tag="small")
                for dt in range(DT):
                    xa = sp.tile([128, 512], F16, tag="xa")
                    nc.scalar.activation(xa[:], xs[:, dt], AF.Abs, scale=1.0)
                    nc.tensor.matmul(mags[:], ones_col[:], xa[:],
                                     start=(dt == 0), stop=(dt == DT - 1),
                                     skip_group_check=True)
                md = sp.tile([1, 512], F32, tag="row")
                nc.scalar.activation(md[:], mags[:], AF.Copy, bias=EPS,
                                     scale=1.0 / D)
                mr = sp.tile([1, 512], F32, tag="row")
                nc.vector.reciprocal_approx_fast(mr[:], md[:])
                rep = sp.tile([128, 512], F32, tag="scr")
                nc.gpsimd.partition_broadcast(rep[:], mr[:])
                for dt in range(DT):
                    nc.vector.tensor_tensor(xn[:, dt, sl], xs[:, dt], rep[:],
                                            ALU.mult)

            def sigpipe(s_ps, w4_out, mb):
                a = sp.tile([128, 512], F32, tag="scr")
                nc.scalar.activation(a[:], s_ps[:], AF.Abs, scale=1.0)
                d = sp.tile([128, 512], F32, tag="scr")
                nc.vector.tensor_scalar(d[:], a[:], scalar1=1.0, scalar2=None,
                                        op0=ALU.add, op1=ALU.bypass)
                r = sp.tile([128, 512], F32, tag="scr")
                nc.vector.reciprocal_approx_fast(r[:], d[:])
                u = sp.tile([128, 512], F32, tag="scr")
                nc.vector.tensor_tensor(u[:], s_ps[:], r[:], ALU.mult)
                w2 = sp.tile([128, 512], F32, tag="scr")
                nc.scalar.activation(w2[:], u[:], AF.Square, bias=ones_b[:],
                                     scale=1.0)
                if mb is not None:
                    w2m = sp.tile([128, 512], F32, tag="scr")
                    nc.gpsimd.affine_select(w2m[:], w2[:], pattern=[[1, 512]],
                                            base=mb, channel_multiplier=-1,
                                            compare_op=ALU.is_ge, fill=0.0)
                    w2 = w2m
                nc.scalar.activation(w4_out[:], w2[:], AF.Square, scale=1.0)

            def gen_prep(lx, l, s):
                """Recv swiglu delta + layer loads (s==0) + norm + v + qk."""
                if l > 0:
                    recv_x((2 * (l - 1) + 1) * NSTRIP + s, s)
                if s == 0:
                    wqksb = wpool.tile([128, DT, FSH], F16, tag="wqk",
                                       name=f"wqk_{l}")
                    nc.gpsimd.dma_start(
                        wqksb[:], wqkT[l].rearrange("(dt p) f -> p dt f",
                                                    p=128))
                    wv = wvpool.tile([128, DT, VSH], F16, tag="wv",
                                     name=f"wv_{l}")
                    nc.gpsimd.dma_start(
                        wv[:], wvT[l].rearrange("(dt p) f -> p dt f", p=128))
                    wosb = wopool.tile([128, 2, D], F16, tag="wo",
                                       name=f"wo_{l}")
                    nc.gpsimd.dma_start(
                        wosb[:], woT[l].rearrange("(dt p) f -> p dt f", p=128))
                    qa, ka = [], []
                    for h in range(HPC):
                        qa.append(qkpool.tile([66, T], F16, tag=f"qa{h}",
                                              name=f"qa{h}_{l}"))
                        ka.append(qkpool.tile([66, T], F16, tag=f"ka{h}",
                                              name=f"ka{h}_{l}"))
                        nc.sync.dma_start(qa[h][64:66, :], qaug[h])
                        nc.sync.dma_start(ka[h][64:66, :], kaug[h])
                    vaug = apool.tile([128, DT, HPC * 65], F16, tag="vaug",
                                      name=f"vaug_{l}")
                    vau = vaug[:].rearrange("p dt (h c) -> p dt h c", h=HPC)
                    nc.vector.memset(vau[:, :, :, 64:65], 1.0)
                    lx.update(wqksb=wqksb, wv=wv, wosb=wosb, qa=qa, ka=ka,
                              vaug=vaug)
                    yield
                sl = slice(s * 512, (s + 1) * 512)
                norm_strip(s)
                yield
                for tt in range(4 * s, 4 * (s + 1)):
                    ps = psA.tile([128, 512], F32, tag="ps")
                    for dt in range(DT):
                        nc.tensor.matmul(ps[:, 0:VSH],
                                         xn[:, dt, tt * 128:(tt + 1) * 128],
                                         lx["wv"][:, dt], start=(dt == 0),
                                         stop=(dt == DT - 1))
                    nc.vector.tensor_copy(
                        lx["vaug"][:, tt]
                        .rearrange("p (h c) -> p h c", h=HPC)[:, :, 0:64],
                        ps[:, 0:VSH].rearrange("p (h c) -> p h c", h=HPC))
                    if tt % 2 == 1:
                        yield
                for ft in range(4):
                    ps = psA.tile([128, 512], F32, tag="ps")
                    for dt in range(DT):
                        nc.tensor.matmul(
                            ps[:], lx["wqksb"][:, dt, ft * 128:(ft + 1) * 128],
                            xn[:, dt, sl], start=(dt == 0), stop=(dt == DT - 1))
                    pair, qk = ft % 2, ft // 2
                    tgt = lx["qa"] if qk == 0 else lx["ka"]
                    nc.scalar.activation(tgt[2 * pair][0:64, sl], ps[0:64, :],
                                         AF.Copy, scale=1.0)
                    nc.scalar.activation(tgt[2 * pair + 1][0:64, sl],
                                         ps[64:128, :], AF.Copy, scale=1.0)
                    yield

            def gen_scores(lx, s):
                """Scores+AV per head; each head's denom tail is emitted
                after the NEXT head's score matmuls (latency hiding)."""
                sl = slice(s * 512, (s + 1) * 512)
                qa, ka, vaug = lx["qa"], lx["ka"], lx["vaug"]
                pend = []

                def denom_tail():
                    h, av = pend.pop(0)
                    dd = sp.tile([1, 512], F32, tag="row")
                    nc.scalar.activation(dd[:], av[64:65, :], AF.Identity,
                                         bias=eps16[:], scale=1.0)
                    dr = sp.tile([1, 512], F32, tag="row")
                    nc.vector.reciprocal_approx_fast(dr[:], dd[:])
                    reps = sp.tile([64, 512], F32, tag="scr")
                    nc.gpsimd.partition_broadcast(reps[:], dr[:])
                    pair, half = h // 2, h % 2
                    nc.vector.tensor_tensor(
                        asb[64 * half:64 * (half + 1), pair, sl],
                        av[0:64, :], reps[:], ALU.mult)

                for h in range(HPC):
                    av = psAcc.tile([65, 512], F32, tag="av",
                                    name=f"av{h}_{s}")
                    tks = _causal_tk(s)
                    for i, tk in enumerate(tks):
                        sc = psSc.tile([128, 512], F32, tag="sc")
                        nc.tensor.matmul(sc[:],
                                         ka[h][:, tk * 128:(tk + 1) * 128],
                                         qa[h][:, sl], start=True, stop=True)
                        w4 = sp.tile([128, 512], F16, tag="w4")
                        sigpipe(sc, w4, _mask_base(tk, s))
                        nc.tensor.matmul(av[:],
                                         vaug[:, tk, h * 65:(h + 1) * 65],
                                         w4[:], start=(i == 0),
                                         stop=(i == len(tks) - 1),
                                         skip_group_check=True)
                        if i == len(tks) - 1 and pend:
                            denom_tail()
                        if i % 4 == 3:
                            yield
                    pend.append((h, av))
                denom_tail()

            def out_proj(lx, s, dl):
                sl = slice(s * 512, (s + 1) * 512)
                xs = xb[s]
                for ot in range(DT):
                    ps = psA.tile([128, 512], F32, tag="ps")
                    for dt in range(2):
                        nc.tensor.matmul(
                            ps[:], lx["wosb"][:, dt, ot * 128:(ot + 1) * 128],
                            asb[:, dt, sl], start=(dt == 0), stop=(dt == 1))
                    nc.vector.scalar_tensor_tensor(dl[:, ot], xs[:, ot], 0.25,
                                                   ps[:], op0=ALU.mult,
                                                   op1=ALU.add)

            def cc_send(idx, dl):
                nc.sync.dma_start(
                    cc_in[idx][:].rearrange("(dt p) t -> p dt t", p=128),
                    dl[:])
                nc.gpsimd.collective_compute(
                    "AllReduce", ALU.add, ins=[cc_in[idx][:]],
                    outs=[cc_out[idx][:]], replica_groups=RG)

            def gen_swiglu(lx, l, s):
                """recv x -> norm -> gate/val + h per ft -> w3 -> cc."""
                recv_x((2 * l) * NSTRIP + s, s)
                if s == 0:
                    wmsb = wmpool.tile([128, DT, 2 * DFF_SH], F16, tag="wm",
                                       name=f"wm_{l}")
                    nc.gpsimd.dma_start(
                        wmsb[:], wmT[l].rearrange("(dt p) f -> p dt f", p=128))
                    w3sb = w3pool.tile([128, NFT_FF, D], F16, tag="w3",
                                       name=f"w3_{l}")
                    nc.gpsimd.dma_start(
                        w3sb[:], w3T[l].rearrange("(dt p) f -> p dt f", p=128))
                    lx.update(wmsb=wmsb, w3sb=w3sb)
                sl = slice(s * 512, (s + 1) * 512)
                norm_strip(s)
                yield
                hsb = hpool.tile([128, NFT_FF, 512], F16, tag="hsb",
                                 name=f"hsb_{l}_{s}")
                for ft in range(NFT_FF):
                    gps = psA.tile([128, 512], F32, tag="ps")
                    vps = psA.tile([128, 512], F32, tag="ps")
                    for dt in range(DT):
                        nc.tensor.matmul(
                            gps[:],
                            lx["wmsb"][:, dt, ft * 128:(ft + 1) * 128],
                            xn[:, dt, sl], start=(dt == 0), stop=(dt == DT - 1))
                    for dt in range(DT):
                        nc.tensor.matmul(
                            vps[:],
                            lx["wmsb"][:, dt, DFF_SH + ft * 128:
                                       DFF_SH + (ft + 1) * 128],
                            xn[:, dt, sl], start=(dt == 0), stop=(dt == DT - 1))
                    ag = sp.tile([128, 512], F32, tag="scr")
                    nc.scalar.activation(ag[:], gps[:], AF.Abs, scale=1.0)
                    d = sp.tile([128, 512], F32, tag="scr")
                    nc.vector.tensor_scalar(d[:], ag[:], scalar1=1.0,
                                            scalar2=None, op0=ALU.add,
                                            op1=ALU.bypass)
                    r = sp.tile([128, 512], F32, tag="scr")
                    nc.vector.reciprocal_approx_fast(r[:], d[:])
                    sq = sp.tile([128, 512], F32, tag="scr")
                    nc.scalar.activation(sq[:], gps[:], AF.Square, scale=1.0)
                    m1 = sp.tile([128, 512], F32, tag="scr")
                    nc.vector.tensor_tensor(m1[:], sq[:], r[:], ALU.mult)
                    m2 = sp.tile([128, 512], F32, tag="scr")
                    nc.vector.tensor_tensor(m2[:], m1[:], gps[:], ALU.add)
                    nc.vector.tensor_tensor(hsb[:, ft], m2[:], vps[:],
                                            ALU.mult)
                    yield
                dl = dpool.tile([128, DT, 512], BF16, tag="dl",
                                name=f"dlm_{l}_{s}")
                xs = xb[s]
                for ot in range(DT):
                    ps = psA.tile([128, 512], F32, tag="ps")
                    for ft in range(NFT_FF):
                        nc.tensor.matmul(
                            ps[:], lx["w3sb"][:, ft, ot * 128:(ot + 1) * 128],
                            hsb[:, ft], start=(ft == 0), stop=(ft == NFT_FF - 1))
                    nc.vector.scalar_tensor_tensor(dl[:, ot], xs[:, ot], 0.25,
                                                   ps[:], op0=ALU.mult,
                                                   op1=ALU.add)
                    if ot % 4 == 3:
                        yield
                cc_send((2 * l + 1) * NSTRIP + s, dl)

            def gen_final(l, s):
                recv_x((2 * l + 1) * NSTRIP + s, s)
                norm_strip(s)
                yield

            def gen_lm(half):
                nvs = (VOC_SH + 511) // 512
                for vs in range(nvs):
                    vw = min(512, VOC_SH - vs * 512)
                    wt = lmw.tile([128, DT, 512], F16, tag="wemb",
                                  name=f"wemb_{half}_{vs}")
                    nc.sync.dma_start(
                        wt[:, :, :vw], membT[:, vs * 512:vs * 512 + vw]
                        .rearrange("(dt p) f -> p dt f", p=128))
                    for tt in range(4 * half, 4 * (half + 1)):
                        ps = psA.tile([128, 512], F32, tag="ps")
                        for dt in range(DT):
                            nc.tensor.matmul(ps[:, :vw],
                                             xn[:, dt, tt * 128:(tt + 1) * 128],
                                             wt[:, dt, :vw],
                                             start=(dt == 0),
                                             stop=(dt == DT - 1))
                        ls = lms.tile([128, 512], F32, tag="lmsb")
                        if tt % 2 == 0:
                            nc.scalar.activation(ls[:, :vw], ps[:, :vw],
                                                 AF.Copy, scale=1.0)
                        else:
                            nc.vector.tensor_copy(ls[:, :vw], ps[:, :vw])
                        nc.sync.dma_start(
                            logits[tt * 128:(tt + 1) * 128,
                                   vs * 512:vs * 512 + vw],
                            ls[:, :vw])
                    yield

            # ---- pipeline ----
            # Sequential stage emission: engine queues are in-order, so each
            # stage whose first op waits on a collective is emitted only
            # after a full independent stage that covers the latency.
            lx = {}
            _run(gen_prep(lx, 0, 0))
            for l in range(L):
                _run(gen_scores(lx, 0))
                dla0 = dpool.tile([128, DT, 512], BF16, tag="dl",
                                  name=f"dla_{l}_0")
                out_proj(lx, 0, dla0)
                cc_send((2 * l) * NSTRIP + 0, dla0)
                _run(gen_prep(lx, l, 1))          # recv(S,l-1,1) under sc(0)
                _run(gen_scores(lx, 1))
                dla1 = dpool.tile([128, DT, 512], BF16, tag="dl",
                                  name=f"dla_{l}_1")
                out_proj(lx, 1, dla1)
                cc_send((2 * l) * NSTRIP + 1, dla1)
                _run(gen_swiglu(lx, l, 0))        # recv(A,0) under sc(1)
                _run(gen_swiglu(lx, l, 1))        # recv(A,1) under swiglu(0)
                if l < L - 1:
                    nx = {}
                    _run(gen_prep(nx, l + 1, 0))  # recv(S,l,0) under swiglu(1)
                    lx = nx
            _run(gen_final(L - 1, 0))
            _run(gen_lm(0))                       # covers recv(S,1)
            _run(gen_final(L - 1, 1))
            _run(gen_lm(1))
    nc.compile()
    return nc


def _prep_inputs(input_ids, emb, qkv_w, out_w, n1_w, n2_w, wm_w, w3_w, fn_w):
    ids = np.asarray(input_ids)
    emb = np.asarray(emb, dtype=np.float32)
    x0 = emb[ids]                                   # [B, T, D]
    iota = np.arange(T, dtype=np.float32)
    qkv_w = np.asarray(qkv_w, dtype=np.float32)
    out_w = np.asarray(out_w, dtype=np.float32)
    wm_w = np.asarray(wm_w, dtype=np.float32)
    w3_w = np.asarray(w3_w, dtype=np.float32)
    n1_w = np.asarray(n1_w, dtype=np.float32)
    n2_w = np.asarray(n2_w, dtype=np.float32)
    fn_w = np.asarray(fn_w, dtype=np.float32)
    import ml_dtypes
    bf16 = ml_dtypes.bfloat16
    per_core = []
    for c in range(NCORES):
        b, r = c // TP, c % TP
        heads = list(range(HPC * r, HPC * r + HPC))
        qa = np.stack([np.stack([-iota,
                                 np.full(T, np.float16(ALIBI[h]), np.float32)])
                       for h in heads])
        ka = np.stack([np.stack([np.full(T, np.float16(ALIBI[h]), np.float32),
                                 iota])
                       for h in heads])
        wqk = np.empty((L, D, FSH), np.float32)
        wv = np.empty((L, D, VSH), np.float32)
        wo = np.empty((L, VSH, D), np.float32)
        wm = np.zeros((L, D, 2 * DFF_SH), np.float32)
        w3 = np.zeros((L, DFF_SH, D), np.float32)
        for l in range(L):
            q3 = qkv_w[l].reshape(3, H, DH, D)
            qrows = q3[0, heads].reshape(VSH, D) * SCALE
            krows = q3[1, heads].reshape(VSH, D)
            vrows = q3[2, heads].reshape(VSH, D)
            n1 = n1_w[l][:, None]
            wqk[l] = np.concatenate([qrows, krows], 0).T * n1
            wv[l] = vrows.T * n1
            ow = out_w[l].reshape(D, H, DH)[:, heads].reshape(D, VSH)
            wo[l] = ow.T
            n2 = n2_w[l][:, None]
            g0, g1 = DFF_SH * r, min(DFF_SH * (r + 1), DFF)
            ng = g1 - g0
            if ng > 0:
                wm[l, :, :ng] = wm_w[l][g0:g1].T * n2
                wm[l, :, DFF_SH:DFF_SH + ng] = wm_w[l][DFF + g0:DFF + g1].T * n2
                w3[l, :ng] = 0.5 * w3_w[l][:, g0:g1].T
        memb = (emb[VOC_SH * r:VOC_SH * (r + 1)] * fn_w[None, :]).T
        f16 = np.float16
        per_core.append(dict(
            x0T=np.ascontiguousarray(x0[b].T).astype(bf16),
            qaug=qa.astype(f16), kaug=ka.astype(f16),
            wqkT=np.ascontiguousarray(wqk).astype(f16),
            wvT=np.ascontiguousarray(wv).astype(f16),
            woT=np.ascontiguousarray(wo).astype(f16),
            wmT=np.ascontiguousarray(wm).astype(f16),
            w3T=np.ascontiguousarray(w3).astype(f16),
            membT=np.ascontiguousarray(memb).astype(f16),
        ))
    return per_core


def kernel(**inputs):
    if "nc" not in _CACHE:
        _CACHE["nc"] = build_nc()
    nc = _CACHE["nc"]
    per_core = _prep_inputs(**inputs)
    res = run_bass_kernel_spmd(nc, per_core, core_ids=list(range(NCORES)),
                               **_CACHE.get("run_kwargs", {}))
    _CACHE["last_result"] = res
    out = np.empty((B, T, V), np.float32)
    for c in range(NCORES):
        b, r = c // TP, c % TP
        out[b, :, VOC_SH * r:VOC_SH * (r + 1)] = res.results[c]["logits"]
    return out


# revision 31
# speedup vs baseline: 1.0692x; 1.0692x over previous
"""AlgebraicTransformerLM on 8 trn2 NeuronCores (Bass/Tile), v3.

DP=2 over batch x TP=4 over heads / d_ffn / vocab. All matmul operands
fp16; ALiBi via integer-index aug rows (exact in fp16). The residual
stream lives as bf16 [d, 512-token-strip] tiles that flow THROUGH the
AllReduce: each core contributes delta + x/4, so the collective output
IS the updated residual and lands by plain DMA (no add ops, no engine
blocking). Emission is interleaved via generators so the elementwise-
bound score phases always have independent matmuls queued behind them.
"""
import contextlib
import math

import numpy as np

import concourse.bacc as bacc
import concourse.mybir as mybir
import concourse.tile as tile
from concourse.bass_utils import run_bass_kernel_spmd

F32 = mybir.dt.float32
F16 = mybir.dt.float16
BF16 = mybir.dt.bfloat16
AF = mybir.ActivationFunctionType
ALU = mybir.AluOpType

B, T, V, D, H, L = 2, 1024, 32000, 1024, 16, 4
DFF = 2730
DH = D // H
SCALE = 1.0 / math.sqrt(DH)
EPS = 1e-6

NCORES = 8
TP = 4
HPC = H // TP               # heads per core (4)
FSH = 2 * DH * HPC          # q+k rows per core (512)
VSH = DH * HPC              # v rows per core (256)
DFF_SH = 768                # padded DFF shard (4*768 >= 2730)
NFT_FF = DFF_SH // 128      # 6
VOC_SH = V // TP            # vocab shard per core (8000)
DT = D // 128               # 8
NSTRIP = T // 512           # 2
RG = [[0, 1, 2, 3], [4, 5, 6, 7]]
ALIBI = [2.0 ** (-8.0 * (i + 1) / H) for i in range(H)]

_CACHE = {}


def _causal_tk(s):
    return list(range((s + 1) * (512 // 128)))


def _mask_base(tk, s):
    base = s * 512 - tk * 128
    return base if tk * 128 + 127 > s * 512 else None


def _run(gen):
    for _ in gen:
        pass


def _weave(*gens):
    live = list(gens)
    while live:
        for g in list(live):
            try:
                next(g)
            except StopIteration:
                live.remove(g)


def build_nc():
    nc = bacc.Bacc("TRN2", target_bir_lowering=False)

    x0T = nc.dram_tensor("x0T", [D, T], BF16, kind="ExternalInput")
    qaug = nc.dram_tensor("qaug", [HPC, 2, T], F16, kind="ExternalInput")
    kaug = nc.dram_tensor("kaug", [HPC, 2, T], F16, kind="ExternalInput")
    wqkT = nc.dram_tensor("wqkT", [L, D, FSH], F16, kind="ExternalInput")
    wvT = nc.dram_tensor("wvT", [L, D, VSH], F16, kind="ExternalInput")
    woT = nc.dram_tensor("woT", [L, VSH, D], F16, kind="ExternalInput")
    wmT = nc.dram_tensor("wmT", [L, D, 2 * DFF_SH], F16, kind="ExternalInput")
    w3T = nc.dram_tensor("w3T", [L, DFF_SH, D], F16, kind="ExternalInput")
    membT = nc.dram_tensor("membT", [D, VOC_SH], F16, kind="ExternalInput")
    logits = nc.dram_tensor("logits", [T, VOC_SH], F32, kind="ExternalOutput")
    NCC = 2 * L * NSTRIP
    cc_in = [nc.dram_tensor(f"cc_in{i}", [D, 512], BF16) for i in range(NCC)]
    cc_out = [nc.dram_tensor(f"cc_out{i}", [D, 512], BF16) for i in range(NCC)]
    ccw_in = nc.dram_tensor("ccw_in", [1, 512], BF16)
    ccw_out = nc.dram_tensor("ccw_out", [1, 512], BF16)

    with tile.TileContext(nc) as tc, contextlib.ExitStack() as ctx:
        persist = ctx.enter_context(tc.tile_pool(name="persist", bufs=1))
        psA = ctx.enter_context(tc.tile_pool(name="psA", bufs=2, space="PSUM"))
        psSc = ctx.enter_context(tc.tile_pool(name="psSc", bufs=3, space="PSUM"))
        psAcc = ctx.enter_context(tc.tile_pool(name="psAcc", bufs=2, space="PSUM"))
        psSm = ctx.enter_context(tc.tile_pool(name="psSm", bufs=1, space="PSUM"))

        xn = persist.tile([128, DT, T], F16, tag="xn")

        of = persist.tile([1, 128], F32, tag="ones_f")
        nc.vector.memset(of[:], 1.0)
        ones_st = persist.tile([1, 128], F16, tag="ones_st")
        nc.vector.tensor_copy(ones_st[:], of[:])
        ocf = persist.tile([128, 1], F32, tag="ones_colf")
        nc.vector.memset(ocf[:], 1.0)
        ones_col = persist.tile([128, 1], F16, tag="ones_col")
        nc.vector.tensor_copy(ones_col[:], ocf[:])
        ones_b = persist.tile([128, 1], F32, tag="ones_bias")
        nc.vector.memset(ones_b[:], 1.0)
        eps16 = persist.tile([1, 1], F32, tag="eps16")
        nc.vector.memset(eps16[:], 16.0 * EPS)
        ccw = persist.tile([1, 512], BF16, tag="ccw")
        nc.vector.memset(ccw[:], 0.0)
        nc.sync.dma_start(ccw_in[:], ccw[:])
        nc.gpsimd.collective_compute("AllReduce", ALU.add, ins=[ccw_in[:]],
                                     outs=[ccw_out[:]], replica_groups=RG)

        # residual stream: one bf16 [128, DT, 512] tile per strip, updated
        # through the collectives. xb[s] is the CURRENT tile for strip s.
        xb = [None, None]

        with tc.tile_pool(name="xpool", bufs=3) as xpool, \
             tc.tile_pool(name="wpool", bufs=1) as wpool, \
             tc.tile_pool(name="wvpool", bufs=1) as wvpool, \
             tc.tile_pool(name="wopool", bufs=1) as wopool, \
             tc.tile_pool(name="wmpool", bufs=1) as wmpool, \
             tc.tile_pool(name="w3pool", bufs=1) as w3pool, \
             tc.tile_pool(name="apool", bufs=1) as apool, \
             tc.tile_pool(name="hpool", bufs=1) as hpool, \
             tc.tile_pool(name="qkpool", bufs=1) as qkpool, \
             tc.tile_pool(name="dpool", bufs=2) as dpool, \
             tc.tile_pool(name="scrpool", bufs=5) as scrpool, \
             tc.tile_pool(name="w4pool", bufs=2) as w4pool, \
             tc.tile_pool(name="rowpool", bufs=2) as rowpool, \
             tc.tile_pool(name="lmw", bufs=2) as lmw, \
             tc.tile_pool(name="lms", bufs=2) as lms:

            class SPool:
                _n = 0

                def tile(self, shape, dtype, tag):
                    SPool._n += 1
                    pool = {"scr": scrpool, "w4": w4pool, "xa": w4pool,
                            "row": rowpool, "rowh": rowpool}[tag]
                    return pool.tile(shape, dtype, tag=tag,
                                     name=f"{tag}_{SPool._n}")

            sp = SPool()
            asb = apool.tile([128, 2, T], F16, tag="asb")

            for s in range(NSTRIP):
                xi = xpool.tile([128, DT, 512], BF16, tag="xs",
                                name=f"x_init{s}")
                nc.sync.dma_start(
                    xi[:], x0T[:, s * 512:(s + 1) * 512]
                    .rearrange("(dt p) t -> p dt t", p=128))
                xb[s] = xi

            def recv_x(idx, s):
                """Updated residual strip arrives from the collective."""
                xs = xpool.tile([128, DT, 512], BF16, tag="xs",
                                name=f"x_{idx}")
                nc.sync.dma_start(
                    xs[:], cc_out[idx][:].rearrange("(dt p) t -> p dt t",
                                                    p=128))
                xb[s] = xs

            def norm_strip(s):
                """xn[:, :, strip] = x / (mean|x| + eps), fp16."""
                sl = slice(s * 512, (s + 1) * 512)
                xs = xb[s]
                mags = psSm.tile([1, 512], F32, tag="small")
                for dt in range(DT):
                    xa = sp.tile([128, 512], F16, tag="xa")
                    nc.scalar.activation(xa[:], xs[:, dt], AF.Abs, scale=1.0)
                    nc.tensor.matmul(mags[:], ones_col[:], xa[:],
                                     start=(dt == 0), stop=(dt == DT - 1),
                                     skip_group_check=True)
                md = sp.tile([1, 512], F32, tag="row")
                nc.scalar.activation(md[:], mags[:], AF.Copy, bias=EPS,
                                     scale=1.0 / D)
                mr = sp.tile([1, 512], F32, tag="row")
                nc.vector.reciprocal_approx_fast(mr[:], md[:])
                mrr = sp.tile([1, 512], F16, tag="rowh")
                nc.scalar.activation(mrr[:], mr[:], AF.Copy, scale=1.0)
                rep = psA.tile([128, 512], F32, tag="ps")
                nc.tensor.matmul(rep[:], ones_st[:], mrr[:], start=True,
                                 stop=True)
                for dt in range(DT):
                    nc.vector.tensor_tensor(xn[:, dt, sl], xs[:, dt], rep[:],
                                            ALU.mult)

            def sigpipe(s_ps, w4_out, W, diag):
                """w4[:, :W] from score psum [:, :W]; if diag, causal-mask
                the first 128 columns (the window starts at the diagonal)."""
                a = sp.tile([128, 512], F32, tag="scr")
                nc.scalar.activation(a[:, :W], s_ps[:, :W], AF.Abs, scale=1.0)
                d = sp.tile([128, 512], F32, tag="scr")
                nc.vector.tensor_scalar(d[:, :W], a[:, :W], scalar1=1.0,
                                        scalar2=None, op0=ALU.add,
                                        op1=ALU.bypass)
                r = sp.tile([128, 512], F32, tag="scr")
                nc.vector.reciprocal_approx_fast(r[:, :W], d[:, :W])
                u = sp.tile([128, 512], F32, tag="scr")
                nc.vector.tensor_tensor(u[:, :W], s_ps[:, :W], r[:, :W],
                                        ALU.mult)
                w2 = sp.tile([128, 512], F32, tag="scr")
                nc.scalar.activation(w2[:, :W], u[:, :W], AF.Square,
                                     bias=ones_b[:], scale=1.0)
                if diag:
                    w2m = sp.tile([128, 512], F32, tag="scr")
                    nc.gpsimd.affine_select(w2m[:, 0:128], w2[:, 0:128],
                                            pattern=[[1, 128]], base=0,
                                            channel_multiplier=-1,
                                            compare_op=ALU.is_ge, fill=0.0)
                    nc.scalar.activation(w4_out[:, 0:128], w2m[:, 0:128],
                                         AF.Square, scale=1.0)
                    if W > 128:
                        nc.scalar.activation(w4_out[:, 128:W], w2[:, 128:W],
                                             AF.Square, scale=1.0)
                else:
                    nc.scalar.activation(w4_out[:, :W], w2[:, :W], AF.Square,
                                         scale=1.0)

            def gen_prep(lx, l, s):
                """Recv swiglu delta + layer loads (s==0) + norm + v + qk."""
                if l > 0:
                    recv_x((2 * (l - 1) + 1) * NSTRIP + s, s)
                if s == 0:
                    wqksb = wpool.tile([128, DT, FSH], F16, tag="wqk",
                                       name=f"wqk_{l}")
                    nc.gpsimd.dma_start(
                        wqksb[:], wqkT[l].rearrange("(dt p) f -> p dt f",
                                                    p=128))
                    wv = wvpool.tile([128, DT, VSH], F16, tag="wv",
                                     name=f"wv_{l}")
                    nc.gpsimd.dma_start(
                        wv[:], wvT[l].rearrange("(dt p) f -> p dt f", p=128))
                    wosb = wopool.tile([128, 2, D], F16, tag="wo",
                                       name=f"wo_{l}")
                    nc.gpsimd.dma_start(
                        wosb[:], woT[l].rearrange("(dt p) f -> p dt f", p=128))
                    qa, ka = [], []
                    for h in range(HPC):
                        qa.append(qkpool.tile([66, T], F16, tag=f"qa{h}",
                                              name=f"qa{h}_{l}"))
                        ka.append(qkpool.tile([66, T], F16, tag=f"ka{h}",
                                              name=f"ka{h}_{l}"))
                        nc.sync.dma_start(qa[h][64:66, :], qaug[h])
                        nc.sync.dma_start(ka[h][64:66, :], kaug[h])
                    vaug = apool.tile([128, DT, HPC * 65], F16, tag="vaug",
                                      name=f"vaug_{l}")
                    vau = vaug[:].rearrange("p dt (h c) -> p dt h c", h=HPC)
                    nc.vector.memset(vau[:, :, :, 64:65], 1.0)
                    lx.update(wqksb=wqksb, wv=wv, wosb=wosb, qa=qa, ka=ka,
                              vaug=vaug)
                    yield
                sl = slice(s * 512, (s + 1) * 512)
                norm_strip(s)
                yield
                for tt in range(4 * s, 4 * (s + 1)):
                    ps = psA.tile([128, 512], F32, tag="ps")
                    for dt in range(DT):
                        nc.tensor.matmul(ps[:, 0:VSH],
                                         xn[:, dt, tt * 128:(tt + 1) * 128],
                                         lx["wv"][:, dt], start=(dt == 0),
                                         stop=(dt == DT - 1))
                    nc.vector.tensor_copy(
                        lx["vaug"][:, tt]
                        .rearrange("p (h c) -> p h c", h=HPC)[:, :, 0:64],
                        ps[:, 0:VSH].rearrange("p (h c) -> p h c", h=HPC))
                    if tt % 2 == 1:
                        yield
                for ft in range(4):
                    ps = psA.tile([128, 512], F32, tag="ps")
                    for dt in range(DT):
                        nc.tensor.matmul(
                            ps[:], lx["wqksb"][:, dt, ft * 128:(ft + 1) * 128],
                            xn[:, dt, sl], start=(dt == 0), stop=(dt == DT - 1))
                    pair, qk = ft % 2, ft // 2
                    tgt = lx["qa"] if qk == 0 else lx["ka"]
                    nc.scalar.activation(tgt[2 * pair][0:64, sl], ps[0:64, :],
                                         AF.Copy, scale=1.0)
                    nc.scalar.activation(tgt[2 * pair + 1][0:64, sl],
                                         ps[64:128, :], AF.Copy, scale=1.0)
                    yield

            def gen_scores(lx, s):
                """Scores+AV per head; each head's denom tail is emitted
                after the NEXT head's score matmuls (latency hiding)."""
                sl = slice(s * 512, (s + 1) * 512)
                qa, ka, vaug = lx["qa"], lx["ka"], lx["vaug"]
                pend = []

                def denom_tail():
                    h, av = pend.pop(0)
                    dd = sp.tile([1, 512], F32, tag="row")
                    nc.scalar.activation(dd[:], av[64:65, :], AF.Identity,
                                         bias=eps16[:], scale=1.0)
                    dr = sp.tile([1, 512], F32, tag="row")
                    nc.vector.reciprocal_approx_fast(dr[:], dd[:])
                    drr = sp.tile([1, 512], F16, tag="rowh")
                    nc.scalar.activation(drr[:], dr[:], AF.Copy, scale=1.0)
                    rep2 = psSm.tile([64, 512], F32, tag="small")
                    nc.tensor.matmul(rep2[:], ones_st[:, 0:64], drr[:],
                                     start=True, stop=True)
                    reps = sp.tile([64, 512], F32, tag="scr")
                    nc.scalar.activation(reps[:], rep2[:], AF.Copy, scale=1.0)
                    pair, half = h // 2, h % 2
                    nc.vector.tensor_tensor(
                        asb[64 * half:64 * (half + 1), pair, sl],
                        av[0:64, :], reps[:], ALU.mult)

                for h in range(HPC):
                    av = psAcc.tile([65, 512], F32, tag="av",
                                    name=f"av{h}_{s}")
                    tks = _causal_tk(s)
                    for i, tk in enumerate(tks):
                        # queries before the diagonal of k-tile tk are
                        # causally dead: trim the window to [off:512]
                        diag = tk * 128 >= s * 512
                        off = tk * 128 - s * 512 if diag else 0
                        W = 512 - off
                        sc = psSc.tile([128, 512], F32, tag="sc")
                        nc.tensor.matmul(sc[:, :W],
                                         ka[h][:, tk * 128:(tk + 1) * 128],
                                         qa[h][:, s * 512 + off:
                                               (s + 1) * 512],
                                         start=True, stop=True)
                        w4 = sp.tile([128, 512], F16, tag="w4")
                        sigpipe(sc, w4, W, diag)
                        nc.tensor.matmul(av[:, off:512],
                                         vaug[:, tk, h * 65:(h + 1) * 65],
                                         w4[:, :W], start=(i == 0),
                                         stop=(i == len(tks) - 1),
                                         skip_group_check=True)
                        if i == len(tks) - 1 and pend:
                            denom_tail()
                        if i % 4 == 3:
                            yield
                    pend.append((h, av))
                denom_tail()

            def out_proj(lx, s, dl):
                sl = slice(s * 512, (s + 1) * 512)
                xs = xb[s]
                for ot in range(DT):
                    ps = psA.tile([128, 512], F32, tag="ps")
                    for dt in range(2):
                        nc.tensor.matmul(
                            ps[:], lx["wosb"][:, dt, ot * 128:(ot + 1) * 128],
                            asb[:, dt, sl], start=(dt == 0), stop=(dt == 1))
                    nc.vector.scalar_tensor_tensor(dl[:, ot], xs[:, ot], 0.25,
                                                   ps[:], op0=ALU.mult,
                                                   op1=ALU.add)

            def cc_send(idx, dl):
                nc.sync.dma_start(
                    cc_in[idx][:].rearrange("(dt p) t -> p dt t", p=128),
                    dl[:])
                nc.gpsimd.collective_compute(
                    "AllReduce", ALU.add, ins=[cc_in[idx][:]],
                    outs=[cc_out[idx][:]], replica_groups=RG)

            def gen_swiglu(lx, l, s):
                """recv x -> norm -> gate/val + h per ft -> w3 -> cc."""
                recv_x((2 * l) * NSTRIP + s, s)
                if s == 0:
                    wmsb = wmpool.tile([128, DT, 2 * DFF_SH], F16, tag="wm",
                                       name=f"wm_{l}")
                    nc.gpsimd.dma_start(
                        wmsb[:], wmT[l].rearrange("(dt p) f -> p dt f", p=128))
                    w3sb = w3pool.tile([128, NFT_FF, D], F16, tag="w3",
                                       name=f"w3_{l}")
                    nc.gpsimd.dma_start(
                        w3sb[:], w3T[l].rearrange("(dt p) f -> p dt f", p=128))
                    lx.update(wmsb=wmsb, w3sb=w3sb)
                sl = slice(s * 512, (s + 1) * 512)
                norm_strip(s)
                yield
                hsb = hpool.tile([128, NFT_FF, 512], F16, tag="hsb",
                                 name=f"hsb_{l}_{s}")
                for ft in range(NFT_FF):
                    gps = psA.tile([128, 512], F32, tag="ps")
                    vps = psA.tile([128, 512], F32, tag="ps")
                    for dt in range(DT):
                        nc.tensor.matmul(
                            gps[:],
                            lx["wmsb"][:, dt, ft * 128:(ft + 1) * 128],
                            xn[:, dt, sl], start=(dt == 0), stop=(dt == DT - 1))
                    for dt in range(DT):
                        nc.tensor.matmul(
                            vps[:],
                            lx["wmsb"][:, dt, DFF_SH + ft * 128:
                                       DFF_SH + (ft + 1) * 128],
                            xn[:, dt, sl], start=(dt == 0), stop=(dt == DT - 1))
                    ag = sp.tile([128, 512], F32, tag="scr")
                    nc.scalar.activation(ag[:], gps[:], AF.Abs, scale=1.0)
                    d = sp.tile([128, 512], F32, tag="scr")
                    nc.vector.tensor_scalar(d[:], ag[:], scalar1=1.0,
                                            scalar2=None, op0=ALU.add,
                                            op1=ALU.bypass)
                    r = sp.tile([128, 512], F32, tag="scr")
                    nc.vector.reciprocal_approx_fast(r[:], d[:])
                    sq = sp.tile([128, 512], F32, tag="scr")
                    nc.scalar.activation(sq[:], gps[:], AF.Square, scale=1.0)
                    m1 = sp.tile([128, 512], F32, tag="scr")
                    nc.vector.tensor_tensor(m1[:], sq[:], r[:], ALU.mult)
                    m2 = sp.tile([128, 512], F32, tag="scr")
                    nc.vector.tensor_tensor(m2[:], m1[:], gps[:], ALU.add)
                    nc.vector.tensor_tensor(hsb[:, ft], m2[:], vps[:],
                                            ALU.mult)
                    yield
                dl = dpool.tile([128, DT, 512], BF16, tag="dl",
                                name=f"dlm_{l}_{s}")
                xs = xb[s]
                for ot in range(DT):
                    ps = psA.tile([128, 512], F32, tag="ps")
                    for ft in range(NFT_FF):
                        nc.tensor.matmul(
                            ps[:], lx["w3sb"][:, ft, ot * 128:(ot + 1) * 128],
                            hsb[:, ft], start=(ft == 0), stop=(ft == NFT_FF - 1))
                    nc.vector.scalar_tensor_tensor(dl[:, ot], xs[:, ot], 0.25,
                                                   ps[:], op0=ALU.mult,
                                                   op1=ALU.add)
                    if ot % 4 == 3:
                        yield
                cc_send((2 * l + 1) * NSTRIP + s, dl)

            def gen_final(l, s):
                recv_x((2 * l + 1) * NSTRIP + s, s)
                norm_strip(s)
                yield

            def gen_lm(half):
                nvs = (VOC_SH + 511) // 512
                for vs in range(nvs):
                    vw = min(512, VOC_SH - vs * 512)
                    wt = lmw.tile([128, DT, 512], F16, tag="wemb",
                                  name=f"wemb_{half}_{vs}")
                    nc.sync.dma_start(
                        wt[:, :, :vw], membT[:, vs * 512:vs * 512 + vw]
                        .rearrange("(dt p) f -> p dt f", p=128))
                    for tt in range(4 * half, 4 * (half + 1)):
                        ps = psA.tile([128, 512], F32, tag="ps")
                        for dt in range(DT):
                            nc.tensor.matmul(ps[:, :vw],
                                             xn[:, dt, tt * 128:(tt + 1) * 128],
                                             wt[:, dt, :vw],
                                             start=(dt == 0),
                                             stop=(dt == DT - 1))
                        ls = lms.tile([128, 512], F32, tag="lmsb")
                        if tt % 2 == 0:
                            nc.scalar.activation(ls[:, :vw], ps[:, :vw],
                                                 AF.Copy, scale=1.0)
                        else:
                            nc.vector.tensor_copy(ls[:, :vw], ps[:, :vw])
                        nc.sync.dma_start(
                            logits[tt * 128:(tt + 1) * 128,
                                   vs * 512:vs * 512 + vw],
                            ls[:, :vw])
                    yield

            # ---- pipeline ----
            # Sequential stage emission: engine queues are in-order, so each
            # stage whose first op waits on a collective is emitted only
            # after a full independent stage that covers the latency.
            lx = {}
            _run(gen_prep(lx, 0, 0))
            for l in range(L):
                _run(gen_scores(lx, 0))
                dla0 = dpool.tile([128, DT, 512], BF16, tag="dl",
                                  name=f"dla_{l}_0")
                out_proj(lx, 0, dla0)
                cc_send((2 * l) * NSTRIP + 0, dla0)
                _run(gen_prep(lx, l, 1))          # recv(S,l-1,1) under sc(0)
                _run(gen_scores(lx, 1))
                dla1 = dpool.tile([128, DT, 512], BF16, tag="dl",
                                  name=f"dla_{l}_1")
                out_proj(lx, 1, dla1)
                cc_send((2 * l) * NSTRIP + 1, dla1)
                _run(gen_swiglu(lx, l, 0))        # recv(A,0) under sc(1)
                _run(gen_swiglu(lx, l, 1))        # recv(A,1) under swiglu(0)
                if l < L - 1:
                    nx = {}
                    _run(gen_prep(nx, l + 1, 0))  # recv(S,l,0) under swiglu(1)
                    lx = nx
            _run(gen_final(L - 1, 0))
            _run(gen_lm(0))                       # covers recv(S,1)
            _run(gen_final(L - 1, 1))
            _run(gen_lm(1))
    nc.compile()
    return nc


def _prep_inputs(input_ids, emb, qkv_w, out_w, n1_w, n2_w, wm_w, w3_w, fn_w):
    ids = np.asarray(input_ids)
    emb = np.asarray(emb, dtype=np.float32)
    x0 = emb[ids]                                   # [B, T, D]
    iota = np.arange(T, dtype=np.float32)
    qkv_w = np.asarray(qkv_w, dtype=np.float32)
    out_w = np.asarray(out_w, dtype=np.float32)
    wm_w = np.asarray(wm_w, dtype=np.float32)
    w3_w = np.asarray(w3_w, dtype=np.float32)
    n1_w = np.asarray(n1_w, dtype=np.float32)
    n2_w = np.asarray(n2_w, dtype=np.float32)
    fn_w = np.asarray(fn_w, dtype=np.float32)
    import ml_dtypes
    bf16 = ml_dtypes.bfloat16
    per_core = []
    for c in range(NCORES):
        b, r = c // TP, c % TP
        heads = list(range(HPC * r, HPC * r + HPC))
        qa = np.stack([np.stack([-iota,
                                 np.full(T, np.float16(ALIBI[h]), np.float32)])
                       for h in heads])
        ka = np.stack([np.stack([np.full(T, np.float16(ALIBI[h]), np.float32),
                                 iota])
                       for h in heads])
        wqk = np.empty((L, D, FSH), np.float32)
        wv = np.empty((L, D, VSH), np.float32)
        wo = np.empty((L, VSH, D), np.float32)
        wm = np.zeros((L, D, 2 * DFF_SH), np.float32)
        w3 = np.zeros((L, DFF_SH, D), np.float32)
        for l in range(L):
            q3 = qkv_w[l].reshape(3, H, DH, D)
            qrows = q3[0, heads].reshape(VSH, D) * SCALE
            krows = q3[1, heads].reshape(VSH, D)
            vrows = q3[2, heads].reshape(VSH, D)
            n1 = n1_w[l][:, None]
            wqk[l] = np.concatenate([qrows, krows], 0).T * n1
            wv[l] = vrows.T * n1
            ow = out_w[l].reshape(D, H, DH)[:, heads].reshape(D, VSH)
            wo[l] = ow.T
            n2 = n2_w[l][:, None]
            g0, g1 = DFF_SH * r, min(DFF_SH * (r + 1), DFF)
            ng = g1 - g0
            if ng > 0:
                wm[l, :, :ng] = wm_w[l][g0:g1].T * n2
                wm[l, :, DFF_SH:DFF_SH + ng] = wm_w[l][DFF + g0:DFF + g1].T * n2
                w3[l, :ng] = 0.5 * w3_w[l][:, g0:g1].T
        memb = (emb[VOC_SH * r:VOC_SH * (r + 1)] * fn_w[None, :]).T
        f16 = np.float16
        per_core.append(dict(
            x0T=np.ascontiguousarray(x0[b].T).astype(bf16),
            qaug=qa.astype(f16), kaug=ka.astype(f16),
            wqkT=np.ascontiguousarray(wqk).astype(f16),
            wvT=np.ascontiguousarray(wv).astype(f16),
            woT=np.ascontiguousarray(wo).astype(f16),
            wmT=np.ascontiguousarray(wm).astype(f16),
            w3T=np.ascontiguousarray(w3).astype(f16),
            membT=np.ascontiguousarray(memb).astype(f16),
        ))
    return per_core


def kernel(**inputs):
    if "nc" not in _CACHE:
        _CACHE["nc"] = build_nc()
    nc = _CACHE["nc"]
    per_core = _prep_inputs(**inputs)
    res = run_bass_kernel_spmd(nc, per_core, core_ids=list(range(NCORES)),
                               **_CACHE.get("run_kwargs", {}))
    _CACHE["last_result"] = res
    out = np.empty((B, T, V), np.float32)
    for c in range(NCORES):
        b, r = c // TP, c % TP
        out[b, :, VOC_SH * r:VOC_SH * (r + 1)] = res.results[c]["logits"]
    return out
